# revision 9
# baseline (speedup 1.0000x reference)
"""BERT-base forward on 8 Trainium2 NeuronCores.

Strategy: data-parallel over batch (B=8 -> 1 sequence per core). Each core
runs the full 12-layer encoder on its [512, 768] sequence in feature-major
layout (features on partitions, tokens on the free dim), so every matmul
takes weights in their natural [Din, Dout] HBM layout as the stationary
operand and activations as the moving operand. Matmuls run in float32r
(TF32 mode: 1 cycle/row at free-dim >= 256 vs 4 cycles/row plain fp32).

Host-side work is limited to shard prep: embedding row gathers (indirect
DMA is unavailable on this stack), transposes to feature-major, and the
final unshard/transpose.

LayerNorm in feature-major: partition-dim stats via ones-vector matmuls,
per-token (free-dim) broadcasts via K=1 ones-row matmuls. Softmax: logits
computed k-major (lhsT = kT chunk, rhs = qT head), exp fused into the PSUM
eviction on the scalar engine, Z via ones-column matmuls, normalization on
the vector engine. GELU (tanh approx) fused into FFN1 eviction.
"""
import sys
sys.path.insert(0, '/opt/trn_rl_repo')
import numpy as np
from contextlib import ExitStack

from concourse import bacc, mybir, tile
from concourse.bass_utils import run_bass_kernel_spmd

FP32 = mybir.dt.float32
FP32R = mybir.dt.float32r
AF = mybir.ActivationFunctionType
ALU = mybir.AluOpType

B, S, D, H, NL, F = 8, 512, 768, 12, 12, 3072
DH = D // H            # 64
DC = D // 128          # 6   d-chunks
FC = F // 128          # 24  f-chunks
EPS = 1e-12
QSCALE = float(H) ** -0.5
N_CORES = 8


def build_bert(n_layers=NL):
    nc = bacc.Bacc("TRN2", target_bir_lowering=False)

    P = {}
    def par(name, shape):
        P[name] = nc.declare_dram_parameter(name, list(shape), FP32, isOutput=False)
        return P[name]

    tokT = par("tokT", (D, S))
    typT = par("typT", (D, S))
    posT = par("posT", (D, S))
    maskneg = par("maskneg", (1, S))
    ones_row = par("ones_row", (1, 128))
    ones_col = par("ones_col", (128, 1))
    emb_ln_g = par("emb_ln_g", (D, 1))
    emb_ln_b = par("emb_ln_b", (D, 1))
    qkv_w = par("qkv_w", (NL, D, 3 * D))
    qkv_b = par("qkv_b", (NL, 3 * D, 1))
    attn_w = par("attn_w", (NL, D, D))
    attn_b = par("attn_b", (NL, D, 1))
    ln1_g = par("ln1_g", (NL, D, 1))
    ln1_b = par("ln1_b", (NL, D, 1))
    ffn1_w = par("ffn1_w", (NL, D, F))
    ffn1_b = par("ffn1_b", (NL, F, 1))
    ffn2_w = par("ffn2_w", (NL, F, D))
    ffn2_b = par("ffn2_b", (NL, D, 1))
    ln2_g = par("ln2_g", (NL, D, 1))
    ln2_b = par("ln2_b", (NL, D, 1))
    pool_w = par("pool_w", (D, D))
    pool_b = par("pool_b", (D, 1))
    hT_out = nc.declare_dram_parameter("hT_out", [D, S], FP32, isOutput=True)
    pooled_out = nc.declare_dram_parameter("pooled_out", [D, 1], FP32, isOutput=True)

    with tile.TileContext(nc) as tc, \
         nc.allow_low_precision(reason="fp32r tiles hold full fp32 bits; TF32 rounding happens in the PE"), \
         ExitStack() as ctx:
        ep = ctx.enter_context

        persist = ep(tc.tile_pool(name="persist", bufs=1))
        wpool = ep(tc.tile_pool(name="wpool", bufs=20))
        vwpool = ep(tc.tile_pool(name="vwpool", bufs=6))
        bpool = ep(tc.tile_pool(name="bpool", bufs=2))
        fbpool = ep(tc.tile_pool(name="fbpool", bufs=2))
        rowpool = ep(tc.tile_pool(name="rowpool", bufs=1))
        sqpool = ep(tc.tile_pool(name="sqpool", bufs=3))
        gpool = ep(tc.tile_pool(name="gpool", bufs=2))
        exppool = ep(tc.tile_pool(name="exppool", bufs=8))
        ps = ep(tc.tile_pool(name="ps", bufs=8, space="PSUM"))

        def psum():
            return ps.tile([128, 512], FP32, tag="ps", name="pst")

        # persistent state (feature-major [feat_chunk 128, S])
        h = [persist.tile([128, S], FP32R, tag=f"h{c}", name=f"h{c}") for c in range(DC)]
        a = [persist.tile([128, S], FP32R, tag=f"a{c}", name=f"a{c}") for c in range(DC)]
        rr = [persist.tile([128, S], FP32R, tag=f"r{c}", name=f"r{c}") for c in range(DC)]
        qT = [persist.tile([128, S], FP32R, tag=f"q{c}", name=f"q{c}") for c in range(DC)]
        kT = [persist.tile([128, S], FP32R, tag=f"k{c}", name=f"k{c}") for c in range(DC)]
        aT = qT  # reuse: qT fully consumed by logits before AV writes aT
        V = [persist.tile([128, D], FP32R, tag=f"v{t}", name=f"v{t}") for t in range(4)]
        onesR = persist.tile([1, 128], FP32R, tag="onesR")
        onesC = persist.tile([128, 1], FP32R, tag="onesC")
        maskrow = persist.tile([1, S], FP32R, tag="maskrow")
        mask_bc = persist.tile([128, S], FP32, tag="maskbc")
        RS = persist.tile([128, S], FP32, tag="RS")
        MRS = persist.tile([128, S], FP32, tag="MRS")

        nc.sync.dma_start(onesR[:], ones_row[:].bitcast(FP32R))
        nc.sync.dma_start(onesC[:], ones_col[:].bitcast(FP32R))
        nc.sync.dma_start(maskrow[:], maskneg[:].bitcast(FP32R))

        pm = psum()
        nc.tensor.matmul(pm[:], onesR[:], maskrow[:], start=True, stop=True)
        nc.scalar.activation(mask_bc[:], pm[:], AF.Copy)

        def col_tile(dram_3d_slice, tag):
            t = bpool.tile([128, 1], FP32, tag=tag, name=tag)
            nc.sync.dma_start(t[:], dram_3d_slice)
            return t

        def layer_norm(inp, g_dram, b_dram, out):
            """inp/out: DC fp32r [128,S] tiles; g/b: [D,1] DRAM APs."""
            gt = [col_tile(g_dram[c * 128:(c + 1) * 128, :], f"lng{c}") for c in range(DC)]
            bt = [col_tile(b_dram[c * 128:(c + 1) * 128, :], f"lnb{c}") for c in range(DC)]
            ps_mean = psum()
            ps_sq = psum()
            for c in range(DC):
                nc.tensor.matmul(ps_mean[0:1, :], onesC[:], inp[c][:],
                                 start=(c == 0), stop=(c == DC - 1))
            for c in range(DC):
                sq = sqpool.tile([128, S], FP32R, tag="sq", name="sq", bufs=2)
                nc.scalar.activation(sq[:], inp[c][:].bitcast(FP32), AF.Square)
                nc.tensor.matmul(ps_sq[0:1, :], onesC[:], sq[:],
                                 start=(c == 0), stop=(c == DC - 1))
            m = rowpool.tile([1, S], FP32, tag="m")
            msq = rowpool.tile([1, S], FP32, tag="msq")
            var = rowpool.tile([1, S], FP32, tag="var")
            sd = rowpool.tile([1, S], FP32, tag="sd")
            rstd = rowpool.tile([1, S], FP32R, tag="rstd")
            mrs = rowpool.tile([1, S], FP32R, tag="mrs")
            nc.vector.tensor_scalar(m[:], ps_mean[0:1, :], 1.0 / D, None, ALU.mult)
            nc.vector.tensor_scalar(msq[:], ps_sq[0:1, :], 1.0 / D, None, ALU.mult)
            nc.vector.tensor_tensor(var[:], m[:], m[:], ALU.mult)
            nc.vector.tensor_tensor(var[:], msq[:], var[:], ALU.subtract)
            nc.vector.tensor_scalar(var[:], var[:], EPS, None, ALU.add)
            nc.scalar.activation(sd[:], var[:], AF.Sqrt)
            nc.vector.reciprocal(rstd[:], sd[:])
            nc.vector.scalar_tensor_tensor(mrs[:], m[:], -1.0, rstd[:].bitcast(FP32),
                                           ALU.mult, ALU.mult)
            ps_rs = psum()
            ps_mrs = psum()
            nc.tensor.matmul(ps_rs[:], onesR[:], rstd[:], start=True, stop=True)
            nc.tensor.matmul(ps_mrs[:], onesR[:], mrs[:], start=True, stop=True)
            nc.scalar.activation(RS[:], ps_rs[:], AF.Copy)
            nc.scalar.activation(MRS[:], ps_mrs[:], AF.Copy)
            for c in range(DC):
                t1 = sqpool.tile([128, S], FP32, tag="lnt", name="lnt", bufs=2)
                nc.vector.scalar_tensor_tensor(t1[:], inp[c][:].bitcast(FP32),
                                               gt[c][:], RS[:], ALU.mult, ALU.mult)
                nc.vector.scalar_tensor_tensor(t1[:], MRS[:], gt[c][:], t1[:],
                                               ALU.mult, ALU.add)
                nc.vector.tensor_scalar(out[c][:], t1[:], bt[c][:], None, ALU.add)

        def wtile(dram_slice, pool=wpool, tag="wt", shape=(128, 128)):
            t = pool.tile(list(shape), FP32R, tag=tag, name=tag)
            nc.sync.dma_start(t[:], dram_slice.bitcast(FP32R))
            return t

        # ---- embeddings: e = tokT + typT + posT, then LN ----
        for c in range(DC):
            sl = slice(c * 128, (c + 1) * 128)
            te = sqpool.tile([128, S], FP32, tag="emb_t", name="te", bufs=1)
            ye = sqpool.tile([128, S], FP32, tag="emb_y", name="ye", bufs=1)
            pe = sqpool.tile([128, S], FP32, tag="emb_p", name="pe", bufs=1)
            nc.sync.dma_start(te[:], tokT[sl, :])
            nc.sync.dma_start(ye[:], typT[sl, :])
            nc.sync.dma_start(pe[:], posT[sl, :])
            nc.vector.tensor_tensor(te[:], te[:], ye[:], ALU.add)
            nc.vector.tensor_tensor(rr[c][:], te[:], pe[:], ALU.add)
        layer_norm(rr, emb_ln_g[:, :], emb_ln_b[:, :], h)

        # ---- encoder layers ----
        for l in range(n_layers):
            # --- QKV: qT/kT feature-major ---
            bqs = []
            for c in range(DC):
                bq = col_tile(qkv_b[l, c * 128:(c + 1) * 128, :], f"bq{c}")
                t = bpool.tile([128, 1], FP32, tag=f"bqs{c}", name=f"bqs{c}")
                nc.vector.tensor_scalar(t[:], bq[:], QSCALE, None, ALU.mult)
                bqs.append(t)
            bks = [col_tile(qkv_b[l, D + c * 128:D + (c + 1) * 128, :], f"bk{c}")
                   for c in range(DC)]
            for m in range(2 * DC):          # 6 q tiles then 6 k tiles
                pq = psum()
                for k6 in range(DC):
                    w = wtile(qkv_w[l, k6 * 128:(k6 + 1) * 128,
                                    m * 128:(m + 1) * 128])
                    nc.tensor.matmul(pq[:], w[:], h[k6][:],
                                     start=(k6 == 0), stop=(k6 == DC - 1))
                if m < DC:
                    nc.scalar.activation(qT[m][:], pq[:], AF.Identity,
                                         bias=bqs[m][:], scale=QSCALE)
                else:
                    nc.scalar.activation(kT[m - DC][:], pq[:], AF.Identity,
                                         bias=bks[m - DC][:])
            # --- V token-major: [tok, vfeat], lhsT = h chunks sliced by token ---
            for nm in range(2):
                vb = fbpool.tile([1, 384], FP32R, tag=f"vb{nm}", name=f"vb{nm}")
                nc.sync.dma_start(
                    vb[:], qkv_b[l, 2 * D + nm * 384:2 * D + (nm + 1) * 384, :]
                    .bitcast(FP32R))
                for tm in range(4):
                    pv = psum()
                    for k6 in range(DC):
                        w = wtile(qkv_w[l, k6 * 128:(k6 + 1) * 128,
                                        2 * D + nm * 384:2 * D + (nm + 1) * 384],
                                  pool=vwpool, tag="vw", shape=(128, 384))
                        nc.tensor.matmul(pv[0:128, 0:384],
                                         h[k6][:, tm * 128:(tm + 1) * 128], w[:],
                                         start=(k6 == 0), stop=False)
                    nc.tensor.matmul(pv[0:128, 0:384], onesR[:], vb[:],
                                     start=False, stop=True)
                    nc.scalar.activation(V[tm][:, nm * 384:(nm + 1) * 384],
                                         pv[0:128, 0:384], AF.Copy)

            # --- attention per head ---
            def emit_logits(hd):
                ht, hr = hd // 2, (hd % 2) * 64
                pls = []
                for kc in range(4):
                    pl = psum()
                    nc.tensor.matmul(
                        pl[:], kT[ht][hr:hr + 64, kc * 128:(kc + 1) * 128],
                        qT[ht][hr:hr + 64, :], start=True, stop=True)
                    pls.append(pl)
                ex = []
                for kc in range(4):
                    e = exppool.tile([128, S], FP32R, tag="exp", name="expt")
                    nc.scalar.activation(e[:], pls[kc][:], AF.Exp)
                    ex.append(e)
                return ex

            def emit_av(hd, ex):
                ht, hr = hd // 2, (hd % 2) * 64
                pz = psum()
                for kc in range(4):
                    nc.tensor.matmul(pz[0:1, :], onesC[:], ex[kc][:],
                                     start=(kc == 0), stop=(kc == 3))
                pav = psum()
                for kc in range(4):
                    nc.tensor.matmul(pav[0:64, :],
                                     V[kc][:, hd * 64:(hd + 1) * 64], ex[kc][:],
                                     start=(kc == 0), stop=(kc == 3))
                rc = rowpool.tile([1, S], FP32R, tag="rc", name="rc", bufs=2)
                nc.vector.reciprocal(rc[:], pz[0:1, :])
                prc = psum()
                nc.tensor.matmul(prc[:], onesR[:], rc[:], start=True, stop=True)
                dst = aT[ht][hr:hr + 64, :]
                nc.scalar.activation(dst, pav[0:64, :], AF.Copy)
                nc.vector.tensor_tensor(dst, dst.bitcast(FP32),
                                        prc[hr:hr + 64, :], ALU.mult)
                nc.vector.tensor_tensor(dst, dst.bitcast(FP32),
                                        mask_bc[hr:hr + 64, :], ALU.add)

            prev = None
            for hd in range(H):
                ex = emit_logits(hd)
                if prev is not None:
                    emit_av(*prev)
                prev = (hd, ex)
            emit_av(*prev)

            # --- attention output projection + residual -> rr ---
            bo = [col_tile(attn_b[l, c * 128:(c + 1) * 128, :], f"bo{c}")
                  for c in range(DC)]
            for m in range(DC):
                po = psum()
                for k6 in range(DC):
                    w = wtile(attn_w[l, k6 * 128:(k6 + 1) * 128,
                                     m * 128:(m + 1) * 128])
                    nc.tensor.matmul(po[:], w[:], aT[k6][:],
                                     start=(k6 == 0), stop=(k6 == DC - 1))
                nc.vector.scalar_tensor_tensor(rr[m][:], po[:], bo[m][:],
                                               h[m][:].bitcast(FP32),
                                               ALU.add, ALU.add)

            layer_norm(rr, ln1_g[l], ln1_b[l], a)

            # --- FFN: f-outer, FFN2 psums held across the f loop ---
            b2 = [col_tile(ffn2_b[l, c * 128:(c + 1) * 128, :], f"b2{c}")
                  for c in range(DC)]
            pf2 = [psum() for _ in range(DC)]
            for f in range(FC):
                b1 = col_tile(ffn1_b[l, f * 128:(f + 1) * 128, :], "b1",)
                p1 = psum()
                for k6 in range(DC):
                    w = wtile(ffn1_w[l, k6 * 128:(k6 + 1) * 128,
                                     f * 128:(f + 1) * 128])
                    nc.tensor.matmul(p1[:], w[:], a[k6][:],
                                     start=(k6 == 0), stop=(k6 == DC - 1))
                g = gpool.tile([128, S], FP32R, tag="g", name="gt")
                nc.scalar.activation(g[:], p1[:], AF.Gelu_apprx_tanh, bias=b1[:])
                for m in range(DC):
                    w = wtile(ffn2_w[l, f * 128:(f + 1) * 128,
                                     m * 128:(m + 1) * 128])
                    nc.tensor.matmul(pf2[m][:], w[:], g[:],
                                     start=(f == 0), stop=(f == FC - 1))
            for m in range(DC):
                nc.vector.scalar_tensor_tensor(rr[m][:], pf2[m][:], b2[m][:],
                                               a[m][:].bitcast(FP32),
                                               ALU.add, ALU.add)

            layer_norm(rr, ln2_g[l], ln2_b[l], h)

        # ---- outputs ----
        for c in range(DC):
            nc.sync.dma_start(hT_out[c * 128:(c + 1) * 128, :],
                              h[c][:].bitcast(FP32))
        for m in range(DC):
            pp = psum()
            for k6 in range(DC):
                w32 = wpool.tile([128, 128], FP32, tag="wt", name="w32")
                nc.sync.dma_start(
                    w32[:], pool_w[k6 * 128:(k6 + 1) * 128, m * 128:(m + 1) * 128])
                nc.tensor.matmul(pp[:, 0:1], w32[:], h[k6][:, 0:1].bitcast(FP32),
                                 start=(k6 == 0), stop=(k6 == DC - 1))
            pb = col_tile(pool_b[m * 128:(m + 1) * 128, :], "pb")
            pot = bpool.tile([128, 1], FP32, tag="pot", name="pot")
            nc.scalar.activation(pot[:], pp[:, 0:1], AF.Tanh, bias=pb[:])
            nc.sync.dma_start(pooled_out[m * 128:(m + 1) * 128, :], pot[:])

    nc.compile()
    return nc


_NC_CACHE = {}


def _get_nc(n_layers=NL):
    if n_layers not in _NC_CACHE:
        _NC_CACHE[n_layers] = build_bert(n_layers)
    return _NC_CACHE[n_layers]


def kernel(input_ids, token_type_ids, position_ids, attention_mask,
           tok_emb, type_emb, pos_emb, emb_ln_g, emb_ln_b,
           qkv_w, qkv_b, attn_w, attn_b, ln1_g, ln1_b,
           ffn1_w, ffn1_b, ffn2_w, ffn2_b, ln2_g, ln2_b,
           pool_w, pool_b, n_layers=NL, _results_hook=None, _trace=False,
           _tmpdir=None):
    inputs = dict(
        input_ids=np.asarray(input_ids), token_type_ids=np.asarray(token_type_ids),
        position_ids=np.asarray(position_ids), attention_mask=np.asarray(attention_mask),
        tok_emb=np.asarray(tok_emb), type_emb=np.asarray(type_emb),
        pos_emb=np.asarray(pos_emb), emb_ln_g=np.asarray(emb_ln_g),
        emb_ln_b=np.asarray(emb_ln_b), qkv_w=np.asarray(qkv_w),
        qkv_b=np.asarray(qkv_b), attn_w=np.asarray(attn_w),
        attn_b=np.asarray(attn_b), ln1_g=np.asarray(ln1_g), ln1_b=np.asarray(ln1_b),
        ffn1_w=np.asarray(ffn1_w), ffn1_b=np.asarray(ffn1_b),
        ffn2_w=np.asarray(ffn2_w), ffn2_b=np.asarray(ffn2_b),
        ln2_g=np.asarray(ln2_g), ln2_b=np.asarray(ln2_b),
        pool_w=np.asarray(pool_w), pool_b=np.asarray(pool_b),
    )
    nc = _get_nc(n_layers)
    C = np.ascontiguousarray

    shared = dict(
        ones_row=np.ones((1, 128), np.float32),
        ones_col=np.ones((128, 1), np.float32),
        emb_ln_g=C(inputs["emb_ln_g"].reshape(D, 1).astype(np.float32)),
        emb_ln_b=C(inputs["emb_ln_b"].reshape(D, 1).astype(np.float32)),
        qkv_w=C(inputs["qkv_w"].astype(np.float32)),
        qkv_b=C(inputs["qkv_b"].reshape(NL, 3 * D, 1).astype(np.float32)),
        attn_w=C(inputs["attn_w"].astype(np.float32)),
        attn_b=C(inputs["attn_b"].reshape(NL, D, 1).astype(np.float32)),
        ln1_g=C(inputs["ln1_g"].reshape(NL, D, 1).astype(np.float32)),
        ln1_b=C(inputs["ln1_b"].reshape(NL, D, 1).astype(np.float32)),
        ffn1_w=C(inputs["ffn1_w"].astype(np.float32)),
        ffn1_b=C(inputs["ffn1_b"].reshape(NL, F, 1).astype(np.float32)),
        ffn2_w=C(inputs["ffn2_w"].astype(np.float32)),
        ffn2_b=C(inputs["ffn2_b"].reshape(NL, D, 1).astype(np.float32)),
        ln2_g=C(inputs["ln2_g"].reshape(NL, D, 1).astype(np.float32)),
        ln2_b=C(inputs["ln2_b"].reshape(NL, D, 1).astype(np.float32)),
        pool_w=C(inputs["pool_w"].astype(np.float32)),
        pool_b=C(inputs["pool_b"].reshape(D, 1).astype(np.float32)),
    )
    posT = C(inputs["pos_emb"][np.asarray(inputs["position_ids"])].T
             .astype(np.float32))
    in_maps = []
    for b in range(B):
        in_maps.append(dict(
            tokT=C(inputs["tok_emb"][np.asarray(inputs["input_ids"][b])].T
                   .astype(np.float32)),
            typT=C(inputs["type_emb"][np.asarray(inputs["token_type_ids"][b])].T
                   .astype(np.float32)),
            posT=posT,
            maskneg=C(((1.0 - inputs["attention_mask"][b]) * -1e9)
                      .reshape(1, S).astype(np.float32)),
            **shared,
        ))

    kw = {}
    if _trace:
        kw = dict(trace=True, tmpdir=_tmpdir)
    res = run_bass_kernel_spmd(nc, in_maps, list(range(N_CORES)), **kw)
    if _results_hook is not None:
        _results_hook(res)
    hs = np.stack([res.results[b]["hT_out"].T for b in range(B)])
    pooled = np.stack([res.results[b]["pooled_out"].reshape(D) for b in range(B)])
    return (hs, pooled)


# revision 11
# speedup vs baseline: 1.8553x; 1.8553x over previous
"""BERT-base forward on 8 Trainium2 NeuronCores.

Strategy: data-parallel over batch (B=8 -> 1 sequence per core). Each core
runs the full 12-layer encoder on its [512, 768] sequence in feature-major
layout (features on partitions, tokens on the free dim), so every matmul
takes weights in their natural [Din, Dout] HBM layout as the stationary
operand and activations as the moving operand. Matmuls run in float32r
(TF32 mode: 1 cycle/row at free-dim >= 256 vs 4 cycles/row plain fp32).

Host-side work is limited to shard prep: embedding row gathers (indirect
DMA is unavailable on this stack), transposes to feature-major, and the
final unshard/transpose.

LayerNorm in feature-major: partition-dim stats via ones-vector matmuls,
per-token (free-dim) broadcasts via K=1 ones-row matmuls. Softmax: logits
computed k-major (lhsT = kT chunk, rhs = qT head), exp fused into the PSUM
eviction on the scalar engine, Z via ones-column matmuls, normalization on
the vector engine. GELU (tanh approx) fused into FFN1 eviction.

Perf notes (from NTFF traces): weight DMAs are issued as wide tiles
([128,384..768]) because each DMA trigger costs ~0.6us on the issuing
engine; small bias/param DMAs go on the GpSimd SWDGE queue; matmul phases
are ordered k-chunk-outer so the PE can start consuming LayerNorm output
chunks as they appear; elementwise work is split between Vector and GpSimd.
"""
import sys
sys.path.insert(0, '/opt/trn_rl_repo')
import numpy as np
from contextlib import ExitStack

from concourse import bacc, mybir, tile
from concourse.bass_utils import run_bass_kernel_spmd

FP32 = mybir.dt.float32
FP32R = mybir.dt.float32r
AF = mybir.ActivationFunctionType
ALU = mybir.AluOpType

B, S, D, H, NL, F = 8, 512, 768, 12, 12, 3072
DH = D // H            # 64
DC = D // 128          # 6   d-chunks
FC = F // 128          # 24  f-chunks
EPS = 1e-12
QSCALE = float(H) ** -0.5
N_CORES = 8


def build_bert(n_layers=NL):
    nc = bacc.Bacc("TRN2", target_bir_lowering=False)

    def par(name, shape):
        return nc.declare_dram_parameter(name, list(shape), FP32, isOutput=False)

    tokT = par("tokT", (D, S))
    typT = par("typT", (D, S))
    posT = par("posT", (D, S))
    maskneg = par("maskneg", (1, S))
    ones_row = par("ones_row", (1, 128))
    ones_col = par("ones_col", (128, 1))
    emb_ln_g = par("emb_ln_g", (D, 1))
    emb_ln_b = par("emb_ln_b", (D, 1))
    qkv_w = par("qkv_w", (NL, D, 3 * D))
    qkv_b = par("qkv_b", (NL, 3 * D, 1))
    attn_w = par("attn_w", (NL, D, D))
    attn_b = par("attn_b", (NL, D, 1))
    ln1_g = par("ln1_g", (NL, D, 1))
    ln1_b = par("ln1_b", (NL, D, 1))
    ffn1_w = par("ffn1_w", (NL, D, F))
    ffn1_b = par("ffn1_b", (NL, F, 1))
    ffn2_w = par("ffn2_w", (NL, F, D))
    ffn2_b = par("ffn2_b", (NL, D, 1))
    ln2_g = par("ln2_g", (NL, D, 1))
    ln2_b = par("ln2_b", (NL, D, 1))
    pool_w = par("pool_w", (D, D))
    pool_b = par("pool_b", (D, 1))
    hT_out = nc.declare_dram_parameter("hT_out", [D, S], FP32, isOutput=True)
    pooled_out = nc.declare_dram_parameter("pooled_out", [D, 1], FP32, isOutput=True)

    with tile.TileContext(nc) as tc, \
         nc.allow_low_precision(reason="fp32r tiles hold full fp32 bits; TF32 rounding happens in the PE"), \
         ExitStack() as ctx:
        ep = ctx.enter_context

        persist = ep(tc.tile_pool(name="persist", bufs=1))
        w768p = ep(tc.tile_pool(name="w768p", bufs=3))
        w256p = ep(tc.tile_pool(name="w256p", bufs=6))
        bpool = ep(tc.tile_pool(name="bpool", bufs=2))
        fbpool = ep(tc.tile_pool(name="fbpool", bufs=2))
        rowpool = ep(tc.tile_pool(name="rowpool", bufs=1))
        sqpool = ep(tc.tile_pool(name="sqpool", bufs=2))
        gpool = ep(tc.tile_pool(name="gpool", bufs=3))
        exppool = ep(tc.tile_pool(name="exppool", bufs=8))
        ps = ep(tc.tile_pool(name="ps", bufs=8, space="PSUM"))

        def psum():
            return ps.tile([128, 512], FP32, tag="ps", name="pst")

        def w768(dram_slice, width=768):
            t = w768p.tile([128, width], FP32R, tag="w768", name="w768t")
            nc.sync.dma_start(t[:], dram_slice.bitcast(FP32R))
            return t

        def w256(dram_slice):
            t = w256p.tile([128, 256], FP32R, tag="w256", name="w256t")
            nc.sync.dma_start(t[:], dram_slice.bitcast(FP32R))
            return t

        # persistent state (feature-major [feat_chunk 128, S])
        h = [persist.tile([128, S], FP32R, tag=f"h{c}", name=f"h{c}") for c in range(DC)]
        a = [persist.tile([128, S], FP32R, tag=f"a{c}", name=f"a{c}") for c in range(DC)]
        rr = [persist.tile([128, S], FP32R, tag=f"r{c}", name=f"r{c}") for c in range(DC)]
        qT = [persist.tile([128, S], FP32R, tag=f"q{c}", name=f"q{c}") for c in range(DC)]
        kT = [persist.tile([128, S], FP32R, tag=f"k{c}", name=f"k{c}") for c in range(DC)]
        aT = qT  # reuse: qT fully consumed by logits before AV writes aT
        V = [persist.tile([128, D], FP32R, tag=f"v{t}", name=f"v{t}") for t in range(4)]
        onesR = persist.tile([1, 128], FP32R, tag="onesR")
        onesC = persist.tile([128, 1], FP32R, tag="onesC")
        maskrow = persist.tile([1, S], FP32R, tag="maskrow")
        mask_bc = persist.tile([128, S], FP32, tag="maskbc")
        RS = persist.tile([128, S], FP32, tag="RS")
        MRS = persist.tile([128, S], FP32, tag="MRS")

        nc.sync.dma_start(onesR[:], ones_row[:].bitcast(FP32R))
        nc.sync.dma_start(onesC[:], ones_col[:].bitcast(FP32R))
        nc.sync.dma_start(maskrow[:], maskneg[:].bitcast(FP32R))

        pm = psum()
        nc.tensor.matmul(pm[:], onesR[:], maskrow[:], start=True, stop=True)
        nc.scalar.activation(mask_bc[:], pm[:], AF.Copy)

        def col_tile(dram_slice, tag):
            t = bpool.tile([128, 1], FP32, tag=tag, name=tag)
            nc.gpsimd.dma_start(t[:], dram_slice)
            return t

        def layer_norm(inp, g_dram, b_dram, out):
            """inp/out: DC fp32r [128,S] tiles; g/b: [D,1] DRAM APs."""
            gt = [col_tile(g_dram[c * 128:(c + 1) * 128, :], f"lng{c}") for c in range(DC)]
            bt = [col_tile(b_dram[c * 128:(c + 1) * 128, :], f"lnb{c}") for c in range(DC)]
            ps_mean = psum()
            ps_sq = psum()
            for c in range(DC):
                nc.tensor.matmul(ps_mean[0:1, :], onesC[:], inp[c][:],
                                 start=(c == 0), stop=(c == DC - 1))
            for c in range(DC):
                sq = sqpool.tile([128, S], FP32R, tag="sq", name="sq")
                nc.scalar.activation(sq[:], inp[c][:].bitcast(FP32), AF.Square)
                nc.tensor.matmul(ps_sq[0:1, :], onesC[:], sq[:],
                                 start=(c == 0), stop=(c == DC - 1))
            m = rowpool.tile([1, S], FP32, tag="m")
            msq = rowpool.tile([1, S], FP32, tag="msq")
            var = rowpool.tile([1, S], FP32, tag="var")
            sd = rowpool.tile([1, S], FP32, tag="sd")
            rstd32 = rowpool.tile([1, S], FP32, tag="rstd32")
            scr = rowpool.tile([1, S], FP32, tag="scr")
            rstd = rowpool.tile([1, S], FP32R, tag="rstd")
            mrs = rowpool.tile([1, S], FP32R, tag="mrs")
            nc.vector.tensor_scalar(m[:], ps_mean[0:1, :], 1.0 / D, None, ALU.mult)
            nc.vector.tensor_scalar(msq[:], ps_sq[0:1, :], 1.0 / D, None, ALU.mult)
            nc.vector.tensor_tensor(var[:], m[:], m[:], ALU.mult)
            nc.vector.tensor_tensor(var[:], msq[:], var[:], ALU.subtract)
            nc.vector.tensor_scalar(var[:], var[:], EPS, None, ALU.add)
            nc.scalar.activation(sd[:], var[:], AF.Sqrt)
            nc.vector.reciprocal_approx_accurate(rstd32[:], sd[:], scr[:])
            nc.vector.tensor_copy(rstd[:], rstd32[:])
            nc.vector.scalar_tensor_tensor(mrs[:], m[:], -1.0, rstd32[:],
                                           ALU.mult, ALU.mult)
            ps_rs = psum()
            ps_mrs = psum()
            nc.tensor.matmul(ps_rs[:], onesR[:], rstd[:], start=True, stop=True)
            nc.tensor.matmul(ps_mrs[:], onesR[:], mrs[:], start=True, stop=True)
            nc.scalar.activation(RS[:], ps_rs[:], AF.Copy)
            nc.scalar.activation(MRS[:], ps_mrs[:], AF.Copy)
            for c in range(DC):
                t1 = sqpool.tile([128, S], FP32, tag="lnt", name="lnt")
                nc.vector.scalar_tensor_tensor(t1[:], inp[c][:].bitcast(FP32),
                                               gt[c][:], RS[:], ALU.mult, ALU.mult)
                nc.vector.scalar_tensor_tensor(t1[:], MRS[:], gt[c][:], t1[:],
                                               ALU.mult, ALU.add)
                nc.vector.tensor_scalar(out[c][:], t1[:], bt[c][:], None, ALU.add)

        # ---- embeddings: e = tokT + typT + posT, then LN ----
        for c in range(DC):
            sl = slice(c * 128, (c + 1) * 128)
            te = sqpool.tile([128, S], FP32, tag="emb_t", name="te", bufs=1)
            ye = sqpool.tile([128, S], FP32, tag="emb_y", name="ye", bufs=1)
            pe = sqpool.tile([128, S], FP32, tag="emb_p", name="pe", bufs=1)
            nc.sync.dma_start(te[:], tokT[sl, :])
            nc.sync.dma_start(ye[:], typT[sl, :])
            nc.sync.dma_start(pe[:], posT[sl, :])
            nc.gpsimd.tensor_tensor(te[:], te[:], ye[:], ALU.add)
            nc.vector.tensor_tensor(rr[c][:], te[:], pe[:], ALU.add)
        layer_norm(rr, emb_ln_g[:, :], emb_ln_b[:, :], h)

        # ---- encoder layers ----
        for l in range(n_layers):
            # per-layer bias/param tiles (SWDGE queue, prefetched early)
            bqs = []
            for c in range(DC):
                bq = col_tile(qkv_b[l, c * 128:(c + 1) * 128, :], f"bq{c}")
                t = bpool.tile([128, 1], FP32, tag=f"bqs{c}", name=f"bqs{c}")
                nc.vector.tensor_scalar(t[:], bq[:], QSCALE, None, ALU.mult)
                bqs.append(t)
            bks = [col_tile(qkv_b[l, D + c * 128:D + (c + 1) * 128, :], f"bk{c}")
                   for c in range(DC)]
            bo = [col_tile(attn_b[l, c * 128:(c + 1) * 128, :], f"bo{c}")
                  for c in range(DC)]
            b2 = [col_tile(ffn2_b[l, c * 128:(c + 1) * 128, :], f"b2{c}")
                  for c in range(DC)]
            vb = []
            for nm in range(2):
                t = fbpool.tile([1, 384], FP32R, tag=f"vb{nm}", name=f"vb{nm}")
                nc.gpsimd.dma_start(
                    t[:], qkv_b[l, 2 * D + nm * 384:2 * D + (nm + 1) * 384, :]
                    .bitcast(FP32R))
                vb.append(t)

            # --- QKV: qT/kT feature-major; k-outer so LN chunks stream in ---
            for g in range(2):               # g=0 -> q tiles, g=1 -> k tiles
                pq = [psum() for _ in range(DC)]
                for k6 in range(DC):
                    w = w768(qkv_w[l, k6 * 128:(k6 + 1) * 128, g * D:(g + 1) * D])
                    for m in range(DC):
                        nc.tensor.matmul(pq[m][:], w[:, m * 128:(m + 1) * 128],
                                         h[k6][:],
                                         start=(k6 == 0), stop=(k6 == DC - 1))
                for m in range(DC):
                    if g == 0:
                        nc.scalar.activation(qT[m][:], pq[m][:], AF.Identity,
                                             bias=bqs[m][:], scale=QSCALE)
                    else:
                        nc.scalar.activation(kT[m][:], pq[m][:], AF.Identity,
                                             bias=bks[m][:])

            # --- V token-major: [tok, vfeat] ---
            for nm in range(2):
                pv = [psum() for _ in range(4)]
                for k6 in range(DC):
                    w = w768(qkv_w[l, k6 * 128:(k6 + 1) * 128,
                                   2 * D + nm * 384:2 * D + (nm + 1) * 384],
                             width=384)
                    for tm in range(4):
                        nc.tensor.matmul(pv[tm][0:128, 0:384],
                                         h[k6][:, tm * 128:(tm + 1) * 128], w[:],
                                         start=(k6 == 0), stop=False)
                for tm in range(4):
                    nc.tensor.matmul(pv[tm][0:128, 0:384], onesR[:], vb[nm][:],
                                     start=False, stop=True)
                    nc.scalar.activation(V[tm][:, nm * 384:(nm + 1) * 384],
                                         pv[tm][0:128, 0:384], AF.Copy)

            # --- attention per head ---
            def emit_logits(hd):
                ht, hr = hd // 2, (hd % 2) * 64
                ex = []
                for kc in range(4):
                    pl = psum()
                    nc.tensor.matmul(
                        pl[:], kT[ht][hr:hr + 64, kc * 128:(kc + 1) * 128],
                        qT[ht][hr:hr + 64, :], start=True, stop=True)
                    e = exppool.tile([128, S], FP32R, tag="exp", name="expt")
                    nc.scalar.activation(e[:], pl[:], AF.Exp)
                    ex.append(e)
                return ex

            def emit_av(hd, ex):
                ht, hr = hd // 2, (hd % 2) * 64
                pz = psum()
                for kc in range(4):
                    nc.tensor.matmul(pz[0:1, :], onesC[:], ex[kc][:],
                                     start=(kc == 0), stop=(kc == 3))
                pav = psum()
                for kc in range(4):
                    nc.tensor.matmul(pav[0:64, :],
                                     V[kc][:, hd * 64:(hd + 1) * 64], ex[kc][:],
                                     start=(kc == 0), stop=(kc == 3))
                rc32 = rowpool.tile([1, S], FP32, tag="rc32", bufs=2, name="rc32")
                scr2 = rowpool.tile([1, S], FP32, tag="scr2", bufs=2, name="scr2")
                nc.vector.tensor_copy(scr2[:], pz[0:1, :])
                nc.vector.reciprocal_approx_fast(rc32[:], scr2[:])
                rc = rowpool.tile([1, S], FP32R, tag="rc", bufs=2, name="rc")
                nc.vector.tensor_copy(rc[:], rc32[:])
                prc = psum()
                nc.tensor.matmul(prc[:], onesR[:], rc[:], start=True, stop=True)
                dst = aT[ht][hr:hr + 64, :]
                nc.scalar.activation(dst, pav[0:64, :], AF.Copy)
                nc.vector.tensor_tensor(dst, dst.bitcast(FP32),
                                        prc[hr:hr + 64, :], ALU.mult)
                nc.gpsimd.tensor_tensor(dst, dst.bitcast(FP32),
                                        mask_bc[hr:hr + 64, :], ALU.add)

            prev = None
            for hd in range(H):
                ex = emit_logits(hd)
                if prev is not None:
                    emit_av(*prev)
                prev = (hd, ex)
            emit_av(*prev)

            # --- attention output projection + residual -> rr (k-outer) ---
            po = [psum() for _ in range(DC)]
            for k6 in range(DC):
                w = w768(attn_w[l, k6 * 128:(k6 + 1) * 128, :])
                for m in range(DC):
                    nc.tensor.matmul(po[m][:], w[:, m * 128:(m + 1) * 128],
                                     aT[k6][:],
                                     start=(k6 == 0), stop=(k6 == DC - 1))
            for m in range(DC):
                nc.vector.scalar_tensor_tensor(rr[m][:], po[m][:], bo[m][:],
                                               h[m][:].bitcast(FP32),
                                               ALU.add, ALU.add)

            layer_norm(rr, ln1_g[l], ln1_b[l], a)

            # --- FFN: f-pairs, k-outer inside; FFN2 psums held across f ---
            pf2 = [psum() for _ in range(DC)]
            for fg in range(FC // 2):
                f0 = 2 * fg
                b1a = col_tile(ffn1_b[l, f0 * 128:(f0 + 1) * 128, :], "b1a")
                b1b = col_tile(ffn1_b[l, (f0 + 1) * 128:(f0 + 2) * 128, :], "b1b")
                p1 = [psum(), psum()]
                for k6 in range(DC):
                    w = w256(ffn1_w[l, k6 * 128:(k6 + 1) * 128,
                                    f0 * 128:(f0 + 2) * 128])
                    for j in range(2):
                        nc.tensor.matmul(p1[j][:], w[:, j * 128:(j + 1) * 128],
                                         a[k6][:],
                                         start=(k6 == 0), stop=(k6 == DC - 1))
                for j in range(2):
                    g = gpool.tile([128, S], FP32R, tag="g", name="gt")
                    nc.scalar.activation(g[:], p1[j][:], AF.Gelu_apprx_tanh,
                                         bias=(b1a if j == 0 else b1b)[:])
                    wf2 = w768(ffn2_w[l, (f0 + j) * 128:(f0 + j + 1) * 128, :])
                    for m in range(DC):
                        nc.tensor.matmul(pf2[m][:], wf2[:, m * 128:(m + 1) * 128],
                                         g[:],
                                         start=(f0 + j == 0),
                                         stop=(f0 + j == FC - 1))
            for m in range(DC):
                nc.vector.scalar_tensor_tensor(rr[m][:], pf2[m][:], b2[m][:],
                                               a[m][:].bitcast(FP32),
                                               ALU.add, ALU.add)

            layer_norm(rr, ln2_g[l], ln2_b[l], h)

        # ---- outputs ----
        for c in range(DC):
            nc.sync.dma_start(hT_out[c * 128:(c + 1) * 128, :],
                              h[c][:].bitcast(FP32))
        pw32 = []
        for k6 in range(DC):
            t = w768p.tile([128, 768], FP32, tag="w768", name="pw32")
            nc.sync.dma_start(t[:], pool_w[k6 * 128:(k6 + 1) * 128, :])
            pw32.append(t)
        for m in range(DC):
            pp = psum()
            for k6 in range(DC):
                nc.tensor.matmul(pp[:, 0:1], pw32[k6][:, m * 128:(m + 1) * 128],
                                 h[k6][:, 0:1].bitcast(FP32),
                                 start=(k6 == 0), stop=(k6 == DC - 1))
            pb = col_tile(pool_b[m * 128:(m + 1) * 128, :], "pb")
            pot = bpool.tile([128, 1], FP32, tag="pot", name="pot")
            nc.scalar.activation(pot[:], pp[:, 0:1], AF.Tanh, bias=pb[:])
            nc.sync.dma_start(pooled_out[m * 128:(m + 1) * 128, :], pot[:])

    nc.compile()
    return nc


_NC_CACHE = {}


def _get_nc(n_layers=NL):
    if n_layers not in _NC_CACHE:
        _NC_CACHE[n_layers] = build_bert(n_layers)
    return _NC_CACHE[n_layers]


def kernel(input_ids, token_type_ids, position_ids, attention_mask,
           tok_emb, type_emb, pos_emb, emb_ln_g, emb_ln_b,
           qkv_w, qkv_b, attn_w, attn_b, ln1_g, ln1_b,
           ffn1_w, ffn1_b, ffn2_w, ffn2_b, ln2_g, ln2_b,
           pool_w, pool_b, n_layers=NL, _results_hook=None, _trace=False,
           _tmpdir=None):
    inputs = dict(
        input_ids=np.asarray(input_ids), token_type_ids=np.asarray(token_type_ids),
        position_ids=np.asarray(position_ids), attention_mask=np.asarray(attention_mask),
        tok_emb=np.asarray(tok_emb), type_emb=np.asarray(type_emb),
        pos_emb=np.asarray(pos_emb), emb_ln_g=np.asarray(emb_ln_g),
        emb_ln_b=np.asarray(emb_ln_b), qkv_w=np.asarray(qkv_w),
        qkv_b=np.asarray(qkv_b), attn_w=np.asarray(attn_w),
        attn_b=np.asarray(attn_b), ln1_g=np.asarray(ln1_g), ln1_b=np.asarray(ln1_b),
        ffn1_w=np.asarray(ffn1_w), ffn1_b=np.asarray(ffn1_b),
        ffn2_w=np.asarray(ffn2_w), ffn2_b=np.asarray(ffn2_b),
        ln2_g=np.asarray(ln2_g), ln2_b=np.asarray(ln2_b),
        pool_w=np.asarray(pool_w), pool_b=np.asarray(pool_b),
    )
    nc = _get_nc(n_layers)
    C = np.ascontiguousarray

    shared = dict(
        ones_row=np.ones((1, 128), np.float32),
        ones_col=np.ones((128, 1), np.float32),
        emb_ln_g=C(inputs["emb_ln_g"].reshape(D, 1).astype(np.float32)),
        emb_ln_b=C(inputs["emb_ln_b"].reshape(D, 1).astype(np.float32)),
        qkv_w=C(inputs["qkv_w"].astype(np.float32)),
        qkv_b=C(inputs["qkv_b"].reshape(NL, 3 * D, 1).astype(np.float32)),
        attn_w=C(inputs["attn_w"].astype(np.float32)),
        attn_b=C(inputs["attn_b"].reshape(NL, D, 1).astype(np.float32)),
        ln1_g=C(inputs["ln1_g"].reshape(NL, D, 1).astype(np.float32)),
        ln1_b=C(inputs["ln1_b"].reshape(NL, D, 1).astype(np.float32)),
        ffn1_w=C(inputs["ffn1_w"].astype(np.float32)),
        ffn1_b=C(inputs["ffn1_b"].reshape(NL, F, 1).astype(np.float32)),
        ffn2_w=C(inputs["ffn2_w"].astype(np.float32)),
        ffn2_b=C(inputs["ffn2_b"].reshape(NL, D, 1).astype(np.float32)),
        ln2_g=C(inputs["ln2_g"].reshape(NL, D, 1).astype(np.float32)),
        ln2_b=C(inputs["ln2_b"].reshape(NL, D, 1).astype(np.float32)),
        pool_w=C(inputs["pool_w"].astype(np.float32)),
        pool_b=C(inputs["pool_b"].reshape(D, 1).astype(np.float32)),
    )
    posT = C(inputs["pos_emb"][np.asarray(inputs["position_ids"])].T
             .astype(np.float32))
    in_maps = []
    for b in range(B):
        in_maps.append(dict(
            tokT=C(inputs["tok_emb"][np.asarray(inputs["input_ids"][b])].T
                   .astype(np.float32)),
            typT=C(inputs["type_emb"][np.asarray(inputs["token_type_ids"][b])].T
                   .astype(np.float32)),
            posT=posT,
            maskneg=C(((1.0 - inputs["attention_mask"][b]) * -1e9)
                      .reshape(1, S).astype(np.float32)),
            **shared,
        ))

    kw = {}
    if _trace:
        kw = dict(trace=True, tmpdir=_tmpdir)
    res = run_bass_kernel_spmd(nc, in_maps, list(range(N_CORES)), **kw)
    if _results_hook is not None:
        _results_hook(res)
    hs = np.stack([res.results[b]["hT_out"].T for b in range(B)])
    pooled = np.stack([res.results[b]["pooled_out"].reshape(D) for b in range(B)])
    return (hs, pooled)
